# revision 15
# baseline (speedup 1.0000x reference)
"""Trainium2 Bass kernel for DynamicPTTopicModeling.

Computes, per batch b (one batch per NeuronCore, 8 cores):
    qg  = relu(qz @ bw.T)            # [R=8192, G=512], contraction over d=1024
    den = max(sum_g qg, 1e-6)        # per-row L1 norm
    msg = (qg @ bw) / den            # [R, D=1024]

Strategy (per core):
  - Process R in 16 "mega-tiles" of 512 rows (4 sub-tiles of 128).
  - PE-transpose qz tiles to get the contraction dim (d) onto partitions.
  - mm1 produces qg TRANSPOSED ([g, p] layout) so mm2 can consume it as the
    stationary operand without further transposes.
  - Row-sums via a PE matmul with a ones-vector stationary; the reciprocal
    scale is applied to the mm2 output via per-partition scalar multiply,
    using 4 tiny PE transposes to move the scale into column layout.
  - All matmuls run in float32r (tf32-like, 1 cycle/row at N=512).
"""
from contextlib import ExitStack

import numpy as np

import concourse.bass as bass
import concourse.tile as tile
from concourse import bacc, mybir
from concourse.bass_utils import run_bass_kernel_spmd
from concourse.masks import make_identity

F32 = mybir.dt.float32
F32R = mybir.dt.float32r
AF = mybir.ActivationFunctionType

B, C, P, D, G = 8, 16, 512, 1024, 512
R = C * P            # 8192 rows per batch
MEGA = 512           # rows per mega-tile
NSUB = MEGA // 128   # 4
NMEGA = R // MEGA    # 16
KD = D // 128        # 8 d-chunks
KG = G // 128        # 4 g-chunks
EPS = 1e-6
N_CORES = 8


def build_kernel():
    nc = bacc.Bacc("TRN2", target_bir_lowering=False)
    # f32r has the same 4-byte np.float32 layout; declaring inputs as f32r
    # makes the PE transposes run at 1.5 cycles/row instead of 2.
    qz_d = nc.dram_tensor("qz", [R, D], F32R, kind="ExternalInput")
    bw_d = nc.dram_tensor("bw", [G, D], F32R, kind="ExternalInput")
    msg_d = nc.dram_tensor("msg", [R, D], F32, kind="ExternalOutput")

    with tile.TileContext(nc) as tc, ExitStack() as ctx:
        const_pool = ctx.enter_context(tc.tile_pool(name="const", bufs=1))
        in_pool = ctx.enter_context(tc.tile_pool(name="inp", bufs=2))
        qzT_pool = ctx.enter_context(tc.tile_pool(name="qzTp", bufs=2))
        qgr_pool = ctx.enter_context(tc.tile_pool(name="qgrp", bufs=2))
        out_pool = ctx.enter_context(tc.tile_pool(name="outp", bufs=2))
        small_pool = ctx.enter_context(tc.tile_pool(name="smallp", bufs=2))
        tr_psum = ctx.enter_context(tc.tile_pool(name="trps", bufs=2, space="PSUM"))
        qg_psum = ctx.enter_context(tc.tile_pool(name="qgps", bufs=2, space="PSUM"))
        msg_psum = ctx.enter_context(tc.tile_pool(name="msgps", bufs=4, space="PSUM"))
        # rowsum/scale psum tiles share the qg pool's slots (tag "qg_ps"):
        # their lifetimes slot cleanly between qg groups, saving 2 banks that
        # msg_psum uses for deeper buffering instead.
        rs_psum = qg_psum
        sc_psum = qg_psum

        identity_f = const_pool.tile([128, 128], F32)
        make_identity(nc, identity_f)
        identity = const_pool.tile([128, 128], F32R)
        nc.vector.tensor_copy(identity, identity_f)
        ones_f = const_pool.tile([128, 1], F32)
        nc.vector.memset(ones_f, 1.0)
        ones_g = const_pool.tile([128, 1], F32R)
        nc.vector.tensor_copy(ones_g, ones_f)
        one_e = const_pool.tile([1, 1], F32)
        nc.vector.memset(one_e, 1.0)

        # bw natural layout [g, d] -> [128, gc, d]; loaded in quarters so the
        # bwT transposes (the first PE work) start as early as possible.
        # f32r round via DVE copy for mm2's moving operand.
        bw_sb = const_pool.tile([128, KG, D], F32R)
        bw_view = bw_d[:].rearrange("(gc p) d -> p gc d", p=128)
        for gc in range(KG):
            nc.sync.dma_start(out=bw_sb[:, gc, :], in_=bw_view[:, gc, :])

        # first qz mega-load queued right behind bw
        qz_in0 = in_pool.tile([128, NSUB, D], F32R, name="qz_in")
        nc.sync.dma_start(
            out=qz_in0,
            in_=qz_d[0:MEGA, :].rearrange("(s p) d -> p s d", p=128),
        )
        bw_r = const_pool.tile([128, KG, D], F32R)
        for gc in range(KG):
            nc.vector.tensor_copy(bw_r[:, gc, :], bw_sb[:, gc, :])

        # bwT[d, g]: [128, k, G]; chunk k holds bw[:, 128k:128(k+1)].T
        # gc-outer so each bw quarter is transposed as soon as it lands.
        bwT_sb = const_pool.tile([128, KD, G], F32R)
        for gc in range(KG):
            for kh in range(2):
                tr_ps = tr_psum.tile([128, MEGA], F32R, name="tr_ps")
                for kk in range(4):
                    k = kh * 4 + kk
                    nc.tensor.matmul(
                        tr_ps[:, kk * 128:(kk + 1) * 128],
                        bw_sb[:, gc, k * 128:(k + 1) * 128],
                        identity,
                        is_transpose=True,
                    )
                nc.vector.tensor_copy(
                    bwT_sb[:, kh * 4:(kh + 1) * 4, gc * 128:(gc + 1) * 128],
                    tr_ps.rearrange("p (kk x) -> p kk x", kk=4),
                )

        for t in range(NMEGA):
            # ---- load 512 rows of qz: [128, s, d] (t=0 preloaded above) ----
            if t == 0:
                qz_in = qz_in0
            else:
                qz_in = in_pool.tile([128, NSUB, D], F32R, name="qz_in")
                nc.sync.dma_start(
                    out=qz_in,
                    in_=qz_d[t * MEGA:(t + 1) * MEGA, :].rearrange(
                        "(s p) d -> p s d", p=128
                    ),
                )

            # ---- transpose to qzT: [128(d), k, MEGA(p)] (f32r) ----
            # copies alternate DVE/ACT so the PSUM->SBUF drain keeps up with
            # the PE transposes (2 tr_ps bufs; copy is the pool-slot release)
            qzT = qzT_pool.tile([128, KD, MEGA], F32R, name="qzT")
            for k in range(KD):
                tr_ps = tr_psum.tile([128, MEGA], F32R, name="tr_ps")
                for s in range(NSUB):
                    nc.tensor.matmul(
                        tr_ps[:, s * 128:(s + 1) * 128],
                        qz_in[:, s, k * 128:(k + 1) * 128],
                        identity,
                        is_transpose=True,
                    )
                if k % 2 == 0:
                    nc.vector.tensor_copy(qzT[:, k, :], tr_ps)
                else:
                    nc.scalar.copy(qzT[:, k, :], tr_ps)

            # ---- mm1: qgT[gc] = sum_k bwT[:,k,gc].T @ qzT[:,k,:]  -> relu ----
            qgr = qgr_pool.tile([128, KG, MEGA], F32R, name="qgr")
            for gc in range(KG):
                qg_ps = qg_psum.tile([128, MEGA], F32, name="qg_ps")
                for k in range(KD):
                    nc.tensor.matmul(
                        qg_ps,
                        bwT_sb[:, k, gc * 128:(gc + 1) * 128],
                        qzT[:, k, :],
                        start=(k == 0),
                        stop=(k == KD - 1),
                    )
                nc.scalar.activation(qgr[:, gc, :], qg_ps, AF.Relu)

            # ---- row sums over g (partition dim) via ones-stationary MM;
            # copied to SBUF so the later PE transposes can read it ----
            rs_ps = rs_psum.tile([1, MEGA], F32, name="rs_ps", tag="qg_ps")
            for gc in range(KG):
                nc.tensor.matmul(
                    rs_ps,
                    ones_g,
                    qgr[:, gc, :],
                    start=(gc == 0),
                    stop=(gc == KG - 1),
                )
            rs_sb = small_pool.tile([1, MEGA], F32, name="rs_sb")
            nc.vector.tensor_copy(rs_sb, rs_ps)

            # ---- mm2: msg[s] = sum_gc qgr[:,gc,s].T @ bw[gc], scaled ----
            # The tiny scale transposes are emitted between mm2 groups 2 and 3
            # so the PE never idles waiting for the rowsum DVE copy; the first
            # scaled copy only needs sc_sb after group 2 anyway.
            msg_sb = out_pool.tile([128, NSUB, D], F32, name="msg_sb")
            sc_sb = None
            pending = []
            for s in range(NSUB):
                for h in range(2):
                    m_ps = msg_psum.tile([128, 512], F32, name="m_ps")
                    for gc in range(KG):
                        nc.tensor.matmul(
                            m_ps,
                            qgr[:, gc, s * 128:(s + 1) * 128],
                            bw_r[:, gc, h * 512:(h + 1) * 512],
                            start=(gc == 0),
                            stop=(gc == KG - 1),
                        )
                    pending.append((s, h, m_ps))

                    if s == 1 and h == 0 and sc_sb is None:
                        # rowsum into column layout via tiny PE transposes,
                        # then max+reciprocal on [128, NSUB]: parallel across
                        # partitions, ~ns instead of a [1,512] reciprocal's µs
                        sc_ps = sc_psum.tile(
                            [128, NSUB], F32, name="sc_ps", tag="qg_ps"
                        )
                        for ss in range(NSUB):
                            nc.tensor.matmul(
                                sc_ps[:, ss:ss + 1],
                                rs_sb[0:1, ss * 128:(ss + 1) * 128],
                                one_e,
                                is_transpose=True,
                            )
                        sc_sb = small_pool.tile([128, NSUB], F32, name="sc_sb")
                        nc.vector.tensor_scalar_max(sc_sb, sc_ps, EPS)
                        nc.vector.reciprocal(sc_sb, sc_sb)
                        for (ps_, hs_, mp_) in pending:
                            dst = msg_sb[:, ps_, hs_ * 512:(hs_ + 1) * 512]
                            if (ps_ * 2 + hs_) % 2 == 0:
                                nc.vector.tensor_scalar_mul(
                                    dst, mp_, sc_sb[:, ps_:ps_ + 1]
                                )
                            else:
                                nc.scalar.mul(dst, mp_, sc_sb[:, ps_:ps_ + 1])
                        pending = []
                    elif sc_sb is not None:
                        dst = msg_sb[:, s, h * 512:(h + 1) * 512]
                        if (s * 2 + h) % 2 == 0:
                            nc.vector.tensor_scalar_mul(dst, m_ps, sc_sb[:, s:s + 1])
                        else:
                            nc.scalar.mul(dst, m_ps, sc_sb[:, s:s + 1])

                # issue the output DMA per 2 subs: finer tail overlap
                if s % 2 == 1:
                    nc.sync.dma_start(
                        out=msg_d[
                            t * MEGA + (s - 1) * 128:t * MEGA + (s + 1) * 128, :
                        ].rearrange("(s p) d -> p s d", p=128),
                        in_=msg_sb[:, s - 1:s + 1, :],
                    )

    nc.compile()
    return nc


_NC_CACHE = None


def _get_nc():
    global _NC_CACHE
    if _NC_CACHE is None:
        _NC_CACHE = build_kernel()
    return _NC_CACHE


def kernel(qz: np.ndarray, binary_weight: np.ndarray) -> np.ndarray:
    qz = np.ascontiguousarray(np.asarray(qz, dtype=np.float32))
    bw = np.ascontiguousarray(np.asarray(binary_weight, dtype=np.float32))
    assert qz.shape == (B, C, P, D), qz.shape
    assert bw.shape == (B, G, D), bw.shape

    nc = _get_nc()
    in_maps = [
        {"qz": qz[i].reshape(R, D), "bw": bw[i]} for i in range(N_CORES)
    ]
    res = run_bass_kernel_spmd(nc, in_maps, core_ids=list(range(N_CORES)))
    out = np.stack(
        [res.results[i]["msg"].reshape(C, P, D) for i in range(N_CORES)], axis=0
    )
    return out
